# revision 9
# baseline (speedup 1.0000x reference)
"""Multi-head attention block (B=16, N=1024, D=768, H=12) on 8 TRN2 NeuronCores.

Strategy: pure data parallelism - 2 batch items per core, no collectives.
Host pre-transposes x to x^T and casts operands to bf16.

Device pipeline (per core), designed around the ACT engine being the global
pacer (exp over N^2*H elements = ~220us at 1 elem/lane/cycle; every other
engine's essential work fits underneath it):
  - scores computed transposed [keys, queries] per (head-pair p, query-half
    hf, key-tile kt): two 64-row-group matmuls run concurrently.
  - exp on ScalarE with fused 1/sqrt(hd) scale; no max subtraction (scores
    are ~N(0,1) by construction).
  - PV is COLUMN-TILED: the two heads' V blocks [128kt, 64] sit in PE column
    groups (0,0)/(0,64) and run concurrently with their own moving ex
    streams, so PV costs 512 cycles/kt instead of 1024 - full PE width.
  - softmax denominators: the ex tiles are folded across kt by a DVE bf16
    add chain, then a single ones-column matmul per (p, hf) contracts the
    128 partitions exactly in f32 (2x512 moving cycles - 24x cheaper than
    the per-kt ones-matmul alternative).
  - normalization: DVE reciprocal + GpSimd partition-broadcast + DVE mult,
    all off the critical path.
  - every non-attention unit (QKV projections, V, output projection) is a
    "filler" drained into the PE slack inside the ACT-paced attention
    windows via a byte-budget queue; DMAs are chunked so the first exp
    fires ~4us into the kernel instead of ~40us.
"""

import sys
import types
import numpy as np
import ml_dtypes
from collections import deque
from contextlib import ExitStack

# --- shim: provide antenv.axon_hooks so trace=True works under axon ---
if "antenv.axon_hooks" not in sys.modules:
    try:
        from trn_agent_boot.trn_boot import _ntff_profile_via_ctypes

        _hooks_mod = types.ModuleType("antenv.axon_hooks")
        _ntff_hook = _ntff_profile_via_ctypes("/opt/axon/libaxon_pjrt.so")
        _hooks_mod.get_axon_ntff_profile_hook = lambda: _ntff_hook
        _hooks_mod.set_axon_ntff_profile_hook = lambda h: None
        sys.modules["antenv.axon_hooks"] = _hooks_mod
    except Exception:
        pass

import concourse.bass as bass
import concourse.tile as tile
from concourse import bacc, mybir
import concourse.bass_utils as bass_utils
from concourse.bass_utils import run_bass_kernel_spmd

bass_utils.upload_artifacts = lambda tmpdir: tmpdir  # no S3 in sandbox

F32 = mybir.dt.float32
BF16 = mybir.dt.bfloat16
EXP = mybir.ActivationFunctionType.Exp
ADD = mybir.AluOpType.add
MULT = mybir.AluOpType.mult

NCORES = 8
B, N, D = 16, 1024, 768
H, HD = 12, 64
BPC = B // NCORES        # batch items per core
ROWS = BPC * N           # 2048
P = 128
KT = D // P              # 6 contraction tiles
NKT = N // P             # 8 attention key tiles
NP = H // 2              # 6 head pairs
SCALE = HD ** -0.5

# filler scheduling: PE slack per ACT-paced attention iteration (~ns)
SLACK_PER_ITER = 650.0
BUDGET_CAP = 2600.0
COST_QK = 1280.0     # 6 matmuls x 512 moving
COST_V = 1920.0      # 12 matmuls x 384 moving
COST_PROJ = 1920.0


def build_kernel():
    nc = bacc.Bacc("TRN2", target_bir_lowering=False, debug=False, num_devices=NCORES)
    xT = nc.dram_tensor("xT", [D, ROWS], BF16, kind="ExternalInput").ap()
    wqkv = nc.dram_tensor("wqkv", [D, 3 * D], BF16, kind="ExternalInput").ap()
    wproj = nc.dram_tensor("wproj", [D, D], BF16, kind="ExternalInput").ap()
    bias = nc.dram_tensor("bias", [P, D], F32, kind="ExternalInput").ap()
    out = nc.dram_tensor("out", [ROWS, D], F32, kind="ExternalOutput").ap()

    with tile.TileContext(nc) as tc, ExitStack() as ctx:
        const = ctx.enter_context(tc.tile_pool(name="const", bufs=1))
        xp = ctx.enter_context(tc.tile_pool(name="xT", bufs=2))
        qkp = ctx.enter_context(tc.tile_pool(name="qkT", bufs=2))
        vp = ctx.enter_context(tc.tile_pool(name="v", bufs=2))
        aop = ctx.enter_context(tc.tile_pool(name="ao", bufs=2))
        exp_p = ctx.enter_context(tc.tile_pool(name="exp", bufs=6))
        accp = ctx.enter_context(tc.tile_pool(name="acc", bufs=2))
        up = ctx.enter_context(tc.tile_pool(name="u", bufs=1))
        smallp = ctx.enter_context(tc.tile_pool(name="small", bufs=1))
        rbcp = ctx.enter_context(tc.tile_pool(name="rbc", bufs=1))
        yp = ctx.enter_context(tc.tile_pool(name="y", bufs=2))
        ps_sc = ctx.enter_context(tc.tile_pool(name="ps_sc", bufs=2, space="PSUM"))
        ps_po = ctx.enter_context(tc.tile_pool(name="ps_po", bufs=2, space="PSUM"))
        ps_mm = ctx.enter_context(tc.tile_pool(name="ps_mm", bufs=2, space="PSUM"))

        # warm the ACT exp table set during the DMA lead-in
        warm = smallp.tile([1, 16], F32, tag="warm")
        nc.vector.memset(warm[:], 0.0)
        warm2 = smallp.tile([1, 16], BF16, tag="warm2")
        nc.scalar.activation(warm2[:], warm[:], EXP, scale=1.0)

        ones_sb = const.tile([P, 1], BF16)
        nc.vector.memset(ones_sb[:], 1.0)

        # --- resident weights / activations, DMA'd in priority order ---
        wqk_sb = const.tile([P, KT, 2 * D], BF16)
        wv_sb = const.tile([P, KT, D], BF16)
        wproj_sb = const.tile([P, KT, D], BF16)
        bias_sb = const.tile([P, D], F32)
        xT_ts = [xp.tile([P, KT, N], BF16, tag="xT", name=f"xT_{b}") for b in range(BPC)]
        qkT_ts = [qkp.tile([P, 2 * KT, N], BF16, tag="qkT", name=f"qkT_{b}") for b in range(BPC)]
        v_ts = [vp.tile([P, NKT, H * HD], BF16, tag="v", name=f"v_{b}") for b in range(BPC)]
        ao_ts = [aop.tile([P, KT, N], BF16, tag="ao", name=f"ao_{b}") for b in range(BPC)]

        def dma_wqk(nt):
            nc.sync.dma_start(
                wqk_sb[:, :, nt * P:(nt + 1) * P],
                wqkv[:, nt * P:(nt + 1) * P].rearrange("(a p) n -> p a n", p=P),
            )

        def dma_xT(b, kt, h):
            nc.sync.dma_start(
                xT_ts[b][:, kt, h * 512:(h + 1) * 512],
                xT[kt * P:(kt + 1) * P, b * N + h * 512:b * N + (h + 1) * 512],
            )

        # first chunks: enough for qk_pair(0, p=0)
        dma_wqk(0)
        dma_wqk(KT)
        for kt in range(KT):
            dma_xT(0, kt, 0)
        # warm the PE clock with dummy matmuls against the first weight chunk
        for w in range(24):
            pmw = ps_mm.tile([P, 512], F32, tag="pm", name=f"pmw_{w}")
            nc.tensor.matmul(
                pmw[:, :256], wqk_sb[:, 0, 0:128], wqk_sb[:, 0, 0:256],
                start=True, stop=True,
            )
        for kt in range(KT):
            nc.sync.dma_start(wv_sb[:, kt, :], wqkv[kt * P:(kt + 1) * P, 2 * D:3 * D])
        for kt in range(KT):
            dma_xT(0, kt, 1)
        for p in range(1, KT):
            dma_wqk(p)
            dma_wqk(KT + p)
        for kt in range(KT):
            dma_xT(1, kt, 0)
            dma_xT(1, kt, 1)
        nc.sync.dma_start(wproj_sb[:], wproj.rearrange("(a p) n -> p a n", p=P))
        nc.sync.dma_start(bias_sb[:], bias)

        # --- work units ---
        def qk_unit(b, nt, hf):
            pm = ps_mm.tile([P, 512], F32, tag="pm")
            for kt in range(KT):
                nc.tensor.matmul(
                    pm[:],
                    wqk_sb[:, kt, nt * P:(nt + 1) * P],
                    xT_ts[b][:, kt, hf * 512:(hf + 1) * 512],
                    start=(kt == 0), stop=(kt == KT - 1),
                )
            nc.vector.tensor_copy(qkT_ts[b][:, nt, hf * 512:(hf + 1) * 512], pm[:])

        def v_unit(b, rt):
            for j in range(2):
                pm = ps_mm.tile([P, 512], F32, tag="pm")
                for kt in range(KT):
                    nc.tensor.matmul(
                        pm[:, :384],
                        xT_ts[b][:, kt, rt * P:(rt + 1) * P],
                        wv_sb[:, kt, j * 384:(j + 1) * 384],
                        start=(kt == 0), stop=(kt == KT - 1),
                    )
                nc.vector.tensor_copy(v_ts[b][:, rt, j * 384:(j + 1) * 384], pm[:, :384])

        def proj_unit(b, rt):
            rows0 = b * N
            y_t = yp.tile([P, D], F32, tag="y")
            for j in range(2):
                pm = ps_mm.tile([P, 512], F32, tag="pm")
                for kt in range(KT):
                    nc.tensor.matmul(
                        pm[:, :384],
                        ao_ts[b][:, kt, rt * P:(rt + 1) * P],
                        wproj_sb[:, kt, j * 384:(j + 1) * 384],
                        start=(kt == 0), stop=(kt == KT - 1),
                    )
                nc.vector.tensor_add(
                    y_t[:, j * 384:(j + 1) * 384], pm[:, :384],
                    bias_sb[:, j * 384:(j + 1) * 384],
                )
            nc.sync.dma_start(out[rows0 + rt * P:rows0 + (rt + 1) * P, :], y_t[:])

        # --- filler queue: non-attention units drained into PE slack ---
        # NOTE: the Tile framework has program-order semantics - a consumer
        # emitted before its producer reads garbage. require() force-drains
        # the queue prefix that produces a given key before its consumer is
        # emitted; the budget only paces, it does not order.
        filler_q = deque()  # items: (cost, key, fn)
        state = {"budget": 3000.0}

        def run_fillers():
            state["budget"] = min(state["budget"] + SLACK_PER_ITER, BUDGET_CAP)
            while filler_q and state["budget"] >= filler_q[0][0]:
                cost, _, fn = filler_q.popleft()
                state["budget"] -= cost
                fn()

        def force_drain():
            while filler_q:
                _, _, fn = filler_q.popleft()
                fn()
            state["budget"] = 3000.0

        def require(keys):
            # pop from the front until no queued item carries one of `keys`
            while any(it[1] in keys for it in filler_q):
                _, _, fn = filler_q.popleft()
                fn()

        def push_qk_pair(b, p):
            for nt in (p, KT + p):
                for hf in range(2):
                    filler_q.append(
                        (COST_QK, ("qk", b, p),
                         lambda b=b, nt=nt, hf=hf: qk_unit(b, nt, hf))
                    )

        def push_v(b, rt):
            filler_q.append((COST_V, ("v", b), lambda b=b, rt=rt: v_unit(b, rt)))

        def push_proj(b, rt):
            filler_q.append((COST_PROJ, ("proj", b), lambda b=b, rt=rt: proj_unit(b, rt)))

        # --- attention ---
        def attn_pair(b, p, on_hf=None, pre_iter=None):
            require({("qk", b, p)} | ({("v", b)} if p == 0 else set()))
            qkT_t, v_t, ao_t = qkT_ts[b], v_ts[b], ao_ts[b]
            for hf in range(2):
                if on_hf is not None:
                    on_hf(hf)
                po = ps_po.tile([P, 512], F32, tag="po")
                acc = None
                for kt in range(NKT):
                    if pre_iter is not None:
                        pre_iter(hf, kt)
                    run_fillers()
                    sc = ps_sc.tile([P, 2, 512], F32, tag="sc")
                    for hs in range(2):
                        qo = hs * HD
                        nc.tensor.matmul(
                            sc[:, hs, :],
                            qkT_t[qo:qo + HD, KT + p, kt * P:(kt + 1) * P],
                            qkT_t[qo:qo + HD, p, hf * 512:(hf + 1) * 512],
                            start=True, stop=True,
                        )
                    ex = exp_p.tile([P, 2, 512], BF16, tag="ex")
                    nc.scalar.activation(ex[:], sc[:], EXP, scale=SCALE)
                    for hs in range(2):
                        nc.tensor.matmul(
                            po[hs * HD:(hs + 1) * HD, :],
                            v_t[:, kt, (2 * p + hs) * HD:(2 * p + hs + 1) * HD],
                            ex[:, hs, :],
                            start=(kt == 0), stop=(kt == NKT - 1),
                        )
                    if acc is None:
                        acc = ex
                    else:
                        a2 = accp.tile([P, 2, 512], BF16, tag="acc")
                        nc.vector.tensor_tensor(a2[:], acc[:], ex[:], ADD)
                        acc = a2
                # epilogue: exact denominators + normalization
                for half in range(2):
                    pmd = ps_mm.tile([P, 512], F32, tag="pm")
                    nc.tensor.matmul(
                        pmd[0:1, :], ones_sb[:], acc[:, half, :],
                        start=True, stop=True,
                    )
                    dsb = smallp.tile([1, 512], F32, tag=f"dsb{half}")
                    nc.vector.tensor_copy(dsb[:], pmd[0:1, :])
                    rec = smallp.tile([1, 512], F32, tag=f"rec{half}")
                    nc.vector.reciprocal_approx_fast(rec[:], dsb[:])
                    rbc = rbcp.tile([HD, 512], F32, tag=f"rbc{half}")
                    nc.gpsimd.partition_broadcast(rbc[:], rec[:])
                    uh = up.tile([HD, 512], F32, tag=f"u{half}")
                    nc.vector.tensor_copy(uh[:], po[half * HD:(half + 1) * HD, :])
                    nc.vector.tensor_tensor(
                        ao_t[half * HD:(half + 1) * HD, p, hf * 512:(hf + 1) * 512],
                        uh[:], rbc[:], MULT,
                    )

        # --- schedule ---
        # serial head: qk for p=0 + first v units, then b0 attention
        for nt in (0, KT):
            for hf in range(2):
                qk_unit(0, nt, hf)
        for rt in range(3):
            v_unit(0, rt)
        for p in range(1, NP):
            push_qk_pair(0, p)
        for rt in range(NKT):
            push_v(1, rt)
        push_qk_pair(1, 0)

        def pre_iter_p0(hf, kt):
            # v0(rt) must be EMITTED before PV(p=0, kt=rt) consumes it;
            # stay 3 tiles ahead during the first pair's hf=0 sweep
            if hf == 0 and kt < NKT - 3:
                v_unit(0, kt + 3)

        for p in range(NP):
            attn_pair(0, p, pre_iter=pre_iter_p0 if p == 0 else None)
        force_drain()

        for p in range(1, NP):
            push_qk_pair(1, p)
        for rt in range(NKT):
            push_proj(0, rt)

        def on_hf_last(hf):
            if hf == 1:
                for rt in range(4):
                    push_proj(1, rt)

        for p in range(NP):
            attn_pair(1, p, on_hf=on_hf_last if p == NP - 1 else None)
        force_drain()
        for rt in range(4, NKT):
            proj_unit(1, rt)

    nc.compile()
    return nc


_NC_CACHE = None


def _get_nc():
    global _NC_CACHE
    if _NC_CACHE is None:
        _NC_CACHE = build_kernel()
    return _NC_CACHE


def make_in_maps(x, W_qkv, W_proj, b_proj):
    x = np.asarray(x, np.float32)
    wq = np.asarray(W_qkv, np.float32).astype(ml_dtypes.bfloat16)
    wp = np.asarray(W_proj, np.float32).astype(ml_dtypes.bfloat16)
    bias = np.ascontiguousarray(
        np.broadcast_to(np.asarray(b_proj, np.float32), (P, D))
    )
    in_maps = []
    for c in range(NCORES):
        xc = x[BPC * c:BPC * (c + 1)].reshape(ROWS, D).T
        in_maps.append({
            "xT": np.ascontiguousarray(xc).astype(ml_dtypes.bfloat16),
            "wqkv": wq, "wproj": wp, "bias": bias,
        })
    return in_maps


def run(x, W_qkv, W_proj, b_proj, trace=False):
    nc = _get_nc()
    in_maps = make_in_maps(x, W_qkv, W_proj, b_proj)
    res = run_bass_kernel_spmd(nc, in_maps, core_ids=list(range(NCORES)), trace=trace)
    y = np.concatenate(
        [res.results[c]["out"].reshape(BPC, N, D) for c in range(NCORES)], axis=0
    )
    return y.astype(np.float32), res


def kernel(x, W_qkv, W_proj, b_proj):
    y, _ = run(x, W_qkv, W_proj, b_proj, trace=False)
    return y


# revision 10
# speedup vs baseline: 1.1180x; 1.1180x over previous
"""Multi-head attention block (B=16, N=1024, D=768, H=12) on 8 TRN2 NeuronCores.

Strategy: pure data parallelism - 2 batch items per core, no collectives.
Host pre-transposes x to x^T and casts operands to bf16.

The ScalarE exp stream (25.2M elems/core = ~220us at 1 elem/lane/cycle) and
the TensorE matmul stream (~246us at 2.4GHz) are the two near-equal floors;
everything else hides underneath. Schedule design:
  - chunked priority DMA (wqk p=0 columns first, then xT halves) so the
    first scores/exp fire ~8us into the kernel instead of ~40us.
  - b0's v units are emitted inline inside the first attention pair's
    iteration loop (PV of pair 0 consumes v tiles as they appear).
  - all other non-attention units (remaining QKV columns, b1's v/qk during
    b0's attention; b0's projection + b1's remaining qk during b1's
    attention; first half of b1's projection inside the last attention
    window) are emitted at PAIR boundaries, i.e. at lower scheduler
    priority than the attention ops - the simulation-based Tile scheduler
    then drops them into true PE idle, and the 2-deep ps_mm pool rotation
    naturally throttles how many filler matmuls can run ahead.
  - scores computed transposed [keys, queries]; per (pair, query-half,
    key-tile): two 64-row-group matmuls run concurrently; exp on ScalarE
    with fused 1/sqrt(hd) scale; a ones column is appended per head so the
    PV matmul also produces the softmax denominators (PE columns 65/128 -
    the 2-head column-tiled alternative was tried and is faster on PE but
    needs a DVE fold for denominators that saturates the vector engine).
  - normalization (copy/broadcast/reciprocal/multiply) runs off the
    critical path on DVE+GpSimd.
"""

import sys
import types
import numpy as np
import ml_dtypes
from collections import deque
from contextlib import ExitStack

# --- shim: provide antenv.axon_hooks so trace=True works under axon ---
if "antenv.axon_hooks" not in sys.modules:
    try:
        from trn_agent_boot.trn_boot import _ntff_profile_via_ctypes

        _hooks_mod = types.ModuleType("antenv.axon_hooks")
        _ntff_hook = _ntff_profile_via_ctypes("/opt/axon/libaxon_pjrt.so")
        _hooks_mod.get_axon_ntff_profile_hook = lambda: _ntff_hook
        _hooks_mod.set_axon_ntff_profile_hook = lambda h: None
        sys.modules["antenv.axon_hooks"] = _hooks_mod
    except Exception:
        pass

import concourse.bass as bass
import concourse.tile as tile
from concourse import bacc, mybir
import concourse.bass_utils as bass_utils
from concourse.bass_utils import run_bass_kernel_spmd

bass_utils.upload_artifacts = lambda tmpdir: tmpdir  # no S3 in sandbox

F32 = mybir.dt.float32
BF16 = mybir.dt.bfloat16
EXP = mybir.ActivationFunctionType.Exp
MULT = mybir.AluOpType.mult

NCORES = 8
B, N, D = 16, 1024, 768
H, HD = 12, 64
BPC = B // NCORES        # batch items per core
ROWS = BPC * N           # 2048
P = 128
KT = D // P              # 6 contraction tiles
NKT = N // P             # 8 attention key tiles
NP = H // 2              # 6 head pairs
SCALE = HD ** -0.5


def build_kernel():
    nc = bacc.Bacc("TRN2", target_bir_lowering=False, debug=False, num_devices=NCORES)
    xT = nc.dram_tensor("xT", [D, ROWS], BF16, kind="ExternalInput").ap()
    wqkv = nc.dram_tensor("wqkv", [D, 3 * D], BF16, kind="ExternalInput").ap()
    wproj = nc.dram_tensor("wproj", [D, D], BF16, kind="ExternalInput").ap()
    bias = nc.dram_tensor("bias", [P, D], F32, kind="ExternalInput").ap()
    out = nc.dram_tensor("out", [ROWS, D], F32, kind="ExternalOutput").ap()

    with tile.TileContext(nc) as tc, ExitStack() as ctx:
        const = ctx.enter_context(tc.tile_pool(name="const", bufs=1))
        xp = ctx.enter_context(tc.tile_pool(name="xT", bufs=2))
        qkp = ctx.enter_context(tc.tile_pool(name="qkT", bufs=2))
        vp = ctx.enter_context(tc.tile_pool(name="v", bufs=2))
        aop = ctx.enter_context(tc.tile_pool(name="ao", bufs=2))
        exp_p = ctx.enter_context(tc.tile_pool(name="exp", bufs=4))
        smallp = ctx.enter_context(tc.tile_pool(name="small", bufs=3))
        yp = ctx.enter_context(tc.tile_pool(name="y", bufs=3))
        ps_sc = ctx.enter_context(tc.tile_pool(name="ps_sc", bufs=2, space="PSUM"))
        ps_out = ctx.enter_context(tc.tile_pool(name="ps_out", bufs=2, space="PSUM"))
        ps_mm = ctx.enter_context(tc.tile_pool(name="ps_mm", bufs=2, space="PSUM"))

        # warm the ACT exp table set during the DMA lead-in
        warm = smallp.tile([1, 16], F32, tag="warm")
        nc.vector.memset(warm[:], 0.0)
        warm2 = smallp.tile([1, 16], BF16, tag="warm2")
        nc.scalar.activation(warm2[:], warm[:], EXP, scale=1.0)

        # --- resident weights / activations, DMA'd in priority order ---
        wqk_sb = const.tile([P, KT, 2 * D], BF16)
        wv_sb = const.tile([P, KT, D], BF16)
        wproj_sb = const.tile([P, KT, D], BF16)
        bias_sb = const.tile([P, D], F32)
        xT_ts = [xp.tile([P, KT, N], BF16, tag="xT", name=f"xT_{b}") for b in range(BPC)]
        qkT_ts = [qkp.tile([P, 2 * KT, N], BF16, tag="qkT", name=f"qkT_{b}") for b in range(BPC)]
        ao_ts = [aop.tile([P, KT, N], BF16, tag="ao", name=f"ao_{b}") for b in range(BPC)]

        def dma_wqk(nt):
            nc.sync.dma_start(
                wqk_sb[:, :, nt * P:(nt + 1) * P],
                wqkv[:, nt * P:(nt + 1) * P].rearrange("(a p) n -> p a n", p=P),
            )

        def dma_xT(b, kt, h):
            nc.sync.dma_start(
                xT_ts[b][:, kt, h * 512:(h + 1) * 512],
                xT[kt * P:(kt + 1) * P, b * N + h * 512:b * N + (h + 1) * 512],
            )

        # first chunks: enough for qk_pair(0, p=0)
        dma_wqk(0)
        dma_wqk(KT)
        for kt in range(KT):
            dma_xT(0, kt, 0)
        # warm the PE clock with dummy matmuls against the first weight chunk
        for w in range(24):
            pmw = ps_mm.tile([P, 512], F32, tag="pm", name=f"pmw_{w}")
            nc.tensor.matmul(
                pmw[:, :256], wqk_sb[:, 0, 0:128], wqk_sb[:, 0, 0:256],
                start=True, stop=True,
            )
        for kt in range(KT):
            nc.sync.dma_start(wv_sb[:, kt, :], wqkv[kt * P:(kt + 1) * P, 2 * D:3 * D])
        for kt in range(KT):
            dma_xT(0, kt, 1)
        for p_ in range(1, KT):
            dma_wqk(p_)
            dma_wqk(KT + p_)
        for kt in range(KT):
            dma_xT(1, kt, 0)
            dma_xT(1, kt, 1)
        nc.sync.dma_start(wproj_sb[:], wproj.rearrange("(a p) n -> p a n", p=P))
        nc.sync.dma_start(bias_sb[:], bias)

        # v tiles carry a ones column per head: PV also produces denominators
        v_ts = []
        for b in range(BPC):
            v_flat = vp.tile([P, NKT, H * (HD + 1)], BF16, tag="v", name=f"v_{b}")
            v_t = v_flat[:].rearrange("q a (h c) -> q a h c", h=H)
            nc.vector.memset(v_t[:, :, :, HD:HD + 1], 1.0)
            v_ts.append(v_t)

        # --- work units ---
        def qk_unit(b, nt, hf):
            pm = ps_mm.tile([P, 512], F32, tag="pm")
            for kt in range(KT):
                nc.tensor.matmul(
                    pm[:],
                    wqk_sb[:, kt, nt * P:(nt + 1) * P],
                    xT_ts[b][:, kt, hf * 512:(hf + 1) * 512],
                    start=(kt == 0), stop=(kt == KT - 1),
                )
            nc.vector.tensor_copy(qkT_ts[b][:, nt, hf * 512:(hf + 1) * 512], pm[:])

        def v_unit(b, rt):
            for j in range(2):
                pm = ps_mm.tile([P, 512], F32, tag="pm")
                for kt in range(KT):
                    nc.tensor.matmul(
                        pm[:, :384],
                        xT_ts[b][:, kt, rt * P:(rt + 1) * P],
                        wv_sb[:, kt, j * 384:(j + 1) * 384],
                        start=(kt == 0), stop=(kt == KT - 1),
                    )
                nc.vector.tensor_copy(
                    v_ts[b][:, rt, j * 6:(j + 1) * 6, 0:HD], pm[:, :384]
                )

        def proj_unit(b, rt):
            rows0 = b * N
            y_t = yp.tile([P, D], F32, tag="y")
            for j in range(2):
                pm = ps_mm.tile([P, 512], F32, tag="pm")
                for kt in range(KT):
                    nc.tensor.matmul(
                        pm[:, :384],
                        ao_ts[b][:, kt, rt * P:(rt + 1) * P],
                        wproj_sb[:, kt, j * 384:(j + 1) * 384],
                        start=(kt == 0), stop=(kt == KT - 1),
                    )
                nc.vector.tensor_add(
                    y_t[:, j * 384:(j + 1) * 384], pm[:, :384],
                    bias_sb[:, j * 384:(j + 1) * 384],
                )
            nc.sync.dma_start(out[rows0 + rt * P:rows0 + (rt + 1) * P, :], y_t[:])

        def qk_pair(b, p):
            for nt in (p, KT + p):
                for hf in range(2):
                    qk_unit(b, nt, hf)

        # --- attention pair: ACT-paced inner loop + normalization epilogue ---
        def attn_pair(b, p, on_hf=None, pre_iter=None):
            qkT_t, v_t, ao_t = qkT_ts[b], v_ts[b], ao_ts[b]
            for hf in range(2):
                if on_hf is not None:
                    on_hf(hf)
                po = [
                    ps_out.tile([HD + 1, 512], F32, tag="po",
                                name=f"po_{b}_{p}_{hf}_{hs}")
                    for hs in range(2)
                ]
                for kt in range(NKT):
                    if pre_iter is not None:
                        pre_iter(hf, kt)
                    sc = ps_sc.tile([P, 2, 512], F32, tag="sc")
                    for hs in range(2):
                        qo = hs * HD
                        nc.tensor.matmul(
                            sc[:, hs, :],
                            qkT_t[qo:qo + HD, KT + p, kt * P:(kt + 1) * P],
                            qkT_t[qo:qo + HD, p, hf * 512:(hf + 1) * 512],
                            start=True, stop=True,
                        )
                    ex = exp_p.tile([P, 2, 512], BF16, tag="ex")
                    nc.scalar.activation(ex[:], sc[:], EXP, scale=SCALE)
                    for hs in range(2):
                        nc.tensor.matmul(
                            po[hs][:],
                            v_t[:, kt, 2 * p + hs, :],
                            ex[:, hs, :],
                            start=(kt == 0), stop=(kt == NKT - 1),
                        )
                for hs in range(2):
                    # single copy releases po; the rest chains off SBUF
                    u65 = smallp.tile([HD + 1, 512], F32, tag="u65")
                    nc.vector.tensor_copy(u65[:], po[hs][:])
                    sums_t = smallp.tile([1, 512], F32, tag="sums")
                    nc.vector.tensor_copy(sums_t[:], u65[HD:HD + 1, :])
                    rbc = smallp.tile([HD, 512], F32, tag="rbc")
                    nc.gpsimd.partition_broadcast(rbc[:], sums_t[:])
                    rec = smallp.tile([HD, 512], F32, tag="rec")
                    nc.vector.reciprocal_approx_fast(rec[:], rbc[:])
                    nc.vector.tensor_tensor(
                        ao_t[hs * HD:(hs + 1) * HD, p, hf * 512:(hf + 1) * 512],
                        u65[0:HD, :], rec[:], MULT,
                    )

        # --- schedule ---
        # serial head: qk for p=0 + first v units; the rest of b0's v is
        # emitted inside pair 0's iteration loop (PV of pair 0 consumes
        # v(rt) at iteration kt=rt; stay 3 ahead)
        qk_pair(0, 0)
        for rt in range(3):
            v_unit(0, rt)

        def pre_iter_p0(hf, kt):
            if hf == 0 and kt < NKT - 3:
                v_unit(0, kt + 3)

        # b0 attention; fillers at pair boundaries (lower priority => the
        # scheduler drops them into PE idle inside the ACT-paced windows)
        for p in range(NP):
            attn_pair(0, p, pre_iter=pre_iter_p0 if p == 0 else None)
            if p < NP - 1:
                qk_pair(0, p + 1)
            if 1 <= p <= 4:
                v_unit(1, 2 * (p - 1))
                v_unit(1, 2 * (p - 1) + 1)
        qk_pair(1, 0)

        def on_hf_last(hf):
            if hf == 1:
                for rt in range(4):
                    proj_unit(1, rt)

        for p in range(NP):
            attn_pair(1, p, on_hf=on_hf_last if p == NP - 1 else None)
            if p < NP - 1:
                qk_pair(1, p + 1)
            if p <= 3:
                proj_unit(0, 2 * p)
                proj_unit(0, 2 * p + 1)
        for rt in range(4):
            proj_unit(1, rt + 4)

    nc.compile()
    return nc


_NC_CACHE = None


def _get_nc():
    global _NC_CACHE
    if _NC_CACHE is None:
        _NC_CACHE = build_kernel()
    return _NC_CACHE


def make_in_maps(x, W_qkv, W_proj, b_proj):
    x = np.asarray(x, np.float32)
    wq = np.asarray(W_qkv, np.float32).astype(ml_dtypes.bfloat16)
    wp = np.asarray(W_proj, np.float32).astype(ml_dtypes.bfloat16)
    bias = np.ascontiguousarray(
        np.broadcast_to(np.asarray(b_proj, np.float32), (P, D))
    )
    in_maps = []
    for c in range(NCORES):
        xc = x[BPC * c:BPC * (c + 1)].reshape(ROWS, D).T
        in_maps.append({
            "xT": np.ascontiguousarray(xc).astype(ml_dtypes.bfloat16),
            "wqkv": wq, "wproj": wp, "bias": bias,
        })
    return in_maps


def run(x, W_qkv, W_proj, b_proj, trace=False):
    nc = _get_nc()
    in_maps = make_in_maps(x, W_qkv, W_proj, b_proj)
    res = run_bass_kernel_spmd(nc, in_maps, core_ids=list(range(NCORES)), trace=trace)
    y = np.concatenate(
        [res.results[c]["out"].reshape(BPC, N, D) for c in range(NCORES)], axis=0
    )
    return y.astype(np.float32), res


def kernel(x, W_qkv, W_proj, b_proj):
    y, _ = run(x, W_qkv, W_proj, b_proj, trace=False)
    return y


# revision 14
# speedup vs baseline: 1.2119x; 1.0839x over previous
"""Multi-head attention block (B=16, N=1024, D=768, H=12) on 8 TRN2 NeuronCores.

Strategy: pure data parallelism - 2 batch items per core, no collectives.
Host pre-transposes x to x^T, pre-arranges W_qkv's q|k columns into the
SBUF-resident [partition, nt, kt, col] layout (so every weight DMA moves
contiguous 1.5KB bursts), and casts operands to bf16.

The ScalarE exp stream (25.2M elems/core, ~214us) and the TensorE matmul
stream (~246us of moving cycles) are near-equal floors. The engine queues
are strict FIFO, so the kernel is built to transmit the ACT pace to the PE
at exactly one point per iteration and never expose a wait anywhere else:
  - scores [keys, queries] per (head-pair, query-half hf, key-tile kt):
    two 64-row-group matmuls run concurrently; exp on ScalarE with fused
    1/sqrt(hd) scale; a ones column per head makes the PV matmul also
    produce the softmax denominators.
  - PV matmuls are software-pipelined TWO iterations behind their exp, so
    they never wait on the ACT engine; only the scores' PSUM-pool rotation
    (2-deep) throttles the PE to the ACT cadence.
  - all non-attention matmuls (QKV columns, V halves, output projection)
    are chopped into single-matmul steps and dripped into each iteration
    by a per-iteration PE-slack budget; require() force-drains any steps
    a pair's inputs depend on before the pair is emitted (the Tile
    framework has program-order semantics).
  - V production is split into j-halves: head-pairs 0-2 need only the
    j=0 half, so pair 0 can start after 4 qk units + a few v half-units.
  - b1's projection is split: 4 row-blocks run inside the last attention
    window, 4 in the tail.
"""

import sys
import types
import numpy as np
import ml_dtypes
from collections import deque
from contextlib import ExitStack

# --- shim: provide antenv.axon_hooks so trace=True works under axon ---
if "antenv.axon_hooks" not in sys.modules:
    try:
        from trn_agent_boot.trn_boot import _ntff_profile_via_ctypes

        _hooks_mod = types.ModuleType("antenv.axon_hooks")
        _ntff_hook = _ntff_profile_via_ctypes("/opt/axon/libaxon_pjrt.so")
        _hooks_mod.get_axon_ntff_profile_hook = lambda: _ntff_hook
        _hooks_mod.set_axon_ntff_profile_hook = lambda h: None
        sys.modules["antenv.axon_hooks"] = _hooks_mod
    except Exception:
        pass

import concourse.bass as bass
import concourse.tile as tile
from concourse import bacc, mybir
import concourse.bass_utils as bass_utils
from concourse.bass_utils import run_bass_kernel_spmd

bass_utils.upload_artifacts = lambda tmpdir: tmpdir  # no S3 in sandbox

F32 = mybir.dt.float32
BF16 = mybir.dt.bfloat16
EXP = mybir.ActivationFunctionType.Exp
MULT = mybir.AluOpType.mult

NCORES = 8
B, N, D = 16, 1024, 768
H, HD = 12, 64
BPC = B // NCORES        # batch items per core
ROWS = BPC * N           # 2048
P = 128
KT = D // P              # 6 contraction tiles
NKT = N // P             # 8 attention key tiles
NP = H // 2              # 6 head pairs
SCALE = HD ** -0.5

SLACK_PER_ITER = 500.0   # PE slack per ACT-paced iteration (ns)
BUDGET_CAP = 1500.0
MM_QK = 215.0            # one 512-moving matmul
MM_VP = 165.0            # one 384-moving matmul


def build_kernel():
    nc = bacc.Bacc("TRN2", target_bir_lowering=False, debug=False, num_devices=NCORES)
    xT = nc.dram_tensor("xT", [D, ROWS], BF16, kind="ExternalInput").ap()
    # host-prearranged: wqk[p, nt*KT*P + kt*P + c] = W_qkv[kt*P + p, nt*P + c]
    wqk = nc.dram_tensor("wqk", [P, 2 * KT * KT * P], BF16, kind="ExternalInput").ap()
    wv = nc.dram_tensor("wv", [D, D], BF16, kind="ExternalInput").ap()
    wproj = nc.dram_tensor("wproj", [D, D], BF16, kind="ExternalInput").ap()
    bias = nc.dram_tensor("bias", [P, D], F32, kind="ExternalInput").ap()
    out = nc.dram_tensor("out", [ROWS, D], F32, kind="ExternalOutput").ap()

    with tile.TileContext(nc) as tc, ExitStack() as ctx:
        const = ctx.enter_context(tc.tile_pool(name="const", bufs=1))
        xp = ctx.enter_context(tc.tile_pool(name="xT", bufs=2))
        qkp = ctx.enter_context(tc.tile_pool(name="qkT", bufs=2))
        vp = ctx.enter_context(tc.tile_pool(name="v", bufs=2))
        aop = ctx.enter_context(tc.tile_pool(name="ao", bufs=2))
        exp_p = ctx.enter_context(tc.tile_pool(name="exp", bufs=4))
        smallp = ctx.enter_context(tc.tile_pool(name="small", bufs=3))
        yp = ctx.enter_context(tc.tile_pool(name="y", bufs=3))
        ps_sc = ctx.enter_context(tc.tile_pool(name="ps_sc", bufs=2, space="PSUM"))
        ps_out = ctx.enter_context(tc.tile_pool(name="ps_out", bufs=2, space="PSUM"))
        ps_mm = ctx.enter_context(tc.tile_pool(name="ps_mm", bufs=2, space="PSUM"))

        # warm the ACT exp table set during the DMA lead-in
        warm = smallp.tile([1, 16], F32, tag="warm")
        nc.vector.memset(warm[:], 0.0)
        warm2 = smallp.tile([1, 16], BF16, tag="warm2")
        nc.scalar.activation(warm2[:], warm[:], EXP, scale=1.0)

        # --- resident weights / activations, DMA'd in priority order ---
        wqk_sb = const.tile([P, 2 * KT, KT, P], BF16)   # [p, nt, kt, c]
        wv_sb = const.tile([P, KT, D], BF16)
        wproj_sb = const.tile([P, KT, D], BF16)
        bias_sb = const.tile([P, D], F32)
        xT_ts = [xp.tile([P, KT, N], BF16, tag="xT", name=f"xT_{b}") for b in range(BPC)]
        qkT_ts = [qkp.tile([P, 2 * KT, N], BF16, tag="qkT", name=f"qkT_{b}") for b in range(BPC)]
        ao_ts = [aop.tile([P, KT, N], BF16, tag="ao", name=f"ao_{b}") for b in range(BPC)]

        def dma_wqk(nt):
            nc.sync.dma_start(
                wqk_sb[:, nt, :, :],
                wqk[:, nt * KT * P:(nt + 1) * KT * P].rearrange(
                    "p (a n) -> p a n", a=KT),
            )

        def dma_xT(b, kt, h):
            nc.sync.dma_start(
                xT_ts[b][:, kt, h * 512:(h + 1) * 512],
                xT[kt * P:(kt + 1) * P, b * N + h * 512:b * N + (h + 1) * 512],
            )

        # first chunks: enough for qk_pair(0, p=0) and early v halves
        dma_wqk(0)
        dma_wqk(KT)
        for kt in range(KT):
            dma_xT(0, kt, 0)
        # a few warm matmuls against the first weight chunk (HAM warm-up)
        for w in range(6):
            pmw = ps_mm.tile([P, 512], F32, tag="pm", name=f"pmw_{w}")
            nc.tensor.matmul(
                pmw[:, :256], wqk_sb[:, 0, 0, :], wqk_sb[:, 0, 0:2, :],
                start=True, stop=True,
            )
        for kt in range(KT):
            nc.sync.dma_start(wv_sb[:, kt, :], wv[kt * P:(kt + 1) * P, :])
        for kt in range(KT):
            dma_xT(0, kt, 1)
        for p_ in range(1, KT):
            dma_wqk(p_)
            dma_wqk(KT + p_)
        for kt in range(KT):
            dma_xT(1, kt, 0)
            dma_xT(1, kt, 1)
        nc.sync.dma_start(wproj_sb[:], wproj.rearrange("(a p) n -> p a n", p=P))
        nc.sync.dma_start(bias_sb[:], bias)

        # v tiles carry a ones column per head: PV also produces denominators
        v_ts = []
        for b in range(BPC):
            v_flat = vp.tile([P, NKT, H * (HD + 1)], BF16, tag="v", name=f"v_{b}")
            v_t = v_flat[:].rearrange("q a (h c) -> q a h c", h=H)
            nc.vector.memset(v_t[:, :, :, HD:HD + 1], 1.0)
            v_ts.append(v_t)

        # --- work units as single-matmul steps ---
        uid = {"n": 0}

        def fresh(tag):
            uid["n"] += 1
            return f"{tag}_{uid['n']}"

        def qk_steps(b, nt, hf):
            st = {}
            def step(kt):
                def f():
                    if kt == 0:
                        st["pm"] = ps_mm.tile([P, 512], F32, tag="pm",
                                              name=fresh("pmqk"))
                    nc.tensor.matmul(
                        st["pm"][:],
                        wqk_sb[:, nt, kt, :],
                        xT_ts[b][:, kt, hf * 512:(hf + 1) * 512],
                        start=(kt == 0), stop=(kt == KT - 1),
                    )
                    if kt == KT - 1:
                        nc.vector.tensor_copy(
                            qkT_ts[b][:, nt, hf * 512:(hf + 1) * 512], st["pm"][:]
                        )
                return f
            return [step(kt) for kt in range(KT)]

        def v_steps(b, rt, j):
            st = {}
            def step(kt):
                def f():
                    if kt == 0:
                        st["pm"] = ps_mm.tile([P, 512], F32, tag="pm",
                                              name=fresh("pmv"))
                    nc.tensor.matmul(
                        st["pm"][:, :384],
                        xT_ts[b][:, kt, rt * P:(rt + 1) * P],
                        wv_sb[:, kt, j * 384:(j + 1) * 384],
                        start=(kt == 0), stop=(kt == KT - 1),
                    )
                    if kt == KT - 1:
                        nc.vector.tensor_copy(
                            v_ts[b][:, rt, j * 6:(j + 1) * 6, 0:HD],
                            st["pm"][:, :384],
                        )
                return f
            return [step(kt) for kt in range(KT)]

        def proj_steps(b, rt):
            rows0 = b * N
            st = {}
            def step(j, kt):
                def f():
                    if j == 0 and kt == 0:
                        st["y"] = yp.tile([P, D], F32, tag="y",
                                          name=fresh("ypj"))
                    if kt == 0:
                        st["pm"] = ps_mm.tile([P, 512], F32, tag="pm",
                                              name=fresh("pmpj"))
                    nc.tensor.matmul(
                        st["pm"][:, :384],
                        ao_ts[b][:, kt, rt * P:(rt + 1) * P],
                        wproj_sb[:, kt, j * 384:(j + 1) * 384],
                        start=(kt == 0), stop=(kt == KT - 1),
                    )
                    if kt == KT - 1:
                        nc.vector.tensor_add(
                            st["y"][:, j * 384:(j + 1) * 384], st["pm"][:, :384],
                            bias_sb[:, j * 384:(j + 1) * 384],
                        )
                        if j == 1:
                            nc.sync.dma_start(
                                out[rows0 + rt * P:rows0 + (rt + 1) * P, :],
                                st["y"][:],
                            )
                return f
            return [step(j, kt) for j in range(2) for kt in range(KT)]

        def emit_unit(steps):
            for s in steps:
                s()

        # --- filler queue (step granularity) + require() ordering guard ---
        filler_q = deque()  # (pe_cost, key, fn)
        state = {"budget": 0.0}

        def push_unit(key, steps, cost):
            for s in steps:
                filler_q.append((cost, key, s))

        def run_fillers():
            state["budget"] = min(state["budget"] + SLACK_PER_ITER, BUDGET_CAP)
            while filler_q and state["budget"] >= filler_q[0][0]:
                cost, _, fn = filler_q.popleft()
                state["budget"] -= cost
                fn()

        def require(keys):
            while any(it[1] in keys for it in filler_q):
                _, _, fn = filler_q.popleft()
                fn()

        def force_drain():
            while filler_q:
                _, _, fn = filler_q.popleft()
                fn()

        # --- attention: PV pipelined 2 iterations behind exp ---
        pv_q = deque()

        def pump_pv(force=False):
            while pv_q and (force or len(pv_q) > 2):
                pv_q.popleft()()

        def epilogue(b, p, hf, po):
            ao_t = ao_ts[b]
            for hs in range(2):
                u65 = smallp.tile([HD + 1, 512], F32, tag="u65")
                nc.vector.tensor_copy(u65[:], po[hs][:])
                sums_t = smallp.tile([1, 512], F32, tag="sums")
                nc.vector.tensor_copy(sums_t[:], u65[HD:HD + 1, :])
                rbc = smallp.tile([HD, 512], F32, tag="rbc")
                nc.gpsimd.partition_broadcast(rbc[:], sums_t[:])
                rec = smallp.tile([HD, 512], F32, tag="rec")
                nc.vector.reciprocal_approx_fast(rec[:], rbc[:])
                nc.vector.tensor_tensor(
                    ao_t[hs * HD:(hs + 1) * HD, p, hf * 512:(hf + 1) * 512],
                    u65[0:HD, :], rec[:], MULT,
                )

        def attn_pair(b, p, pre_iter=None):
            require({("qk", b, p), ("v", b, 0 if p < 3 else 1)})
            qkT_t, v_t = qkT_ts[b], v_ts[b]
            for hf in range(2):
                po = [
                    ps_out.tile([HD + 1, 512], F32, tag="po",
                                name=f"po_{b}_{p}_{hf}_{hs}")
                    for hs in range(2)
                ]
                for kt in range(NKT):
                    if pre_iter is not None:
                        pre_iter(hf, kt)
                    sc = ps_sc.tile([P, 2, 512], F32, tag="sc")
                    for hs in range(2):
                        qo = hs * HD
                        nc.tensor.matmul(
                            sc[:, hs, :],
                            qkT_t[qo:qo + HD, KT + p, kt * P:(kt + 1) * P],
                            qkT_t[qo:qo + HD, p, hf * 512:(hf + 1) * 512],
                            start=True, stop=True,
                        )
                    ex = exp_p.tile([P, 2, 512], BF16, tag="ex")
                    nc.scalar.activation(ex[:], sc[:], EXP, scale=SCALE)

                    def mk_pv(hf=hf, kt=kt, ex=ex, po=po):
                        def f():
                            for hs in range(2):
                                nc.tensor.matmul(
                                    po[hs][:],
                                    v_t[:, kt, 2 * p + hs, :],
                                    ex[:, hs, :],
                                    start=(kt == 0), stop=(kt == NKT - 1),
                                )
                            if kt == NKT - 1:
                                epilogue(b, p, hf, po)
                        return f
                    pv_q.append(mk_pv())
                    pump_pv()
                    run_fillers()

        # --- schedule ---
        # head: qk for pair 0, then v(j=0) halves dripped so PV(p0, kt)
        # (popped at iteration kt+2) always finds v0(rt=kt) already emitted
        for nt in (0, KT):
            for hf in range(2):
                emit_unit(qk_steps(0, nt, hf))
        emit_unit(v_steps(0, 0, 0))
        emit_unit(v_steps(0, 1, 0))

        def pre_iter_p0(hf, kt):
            if hf == 0 and kt < NKT - 2:
                emit_unit(v_steps(0, kt + 2, 0))

        # b0-window fillers (production order matches consumption order)
        for rt in range(NKT):
            push_unit(("v", 0, 1), v_steps(0, rt, 1), MM_VP)
        for p_ in range(1, NP):
            push_unit(("qk", 0, p_), qk_steps(0, p_, 0), MM_QK)
            push_unit(("qk", 0, p_), qk_steps(0, KT + p_, 0), MM_QK)
            push_unit(("qk", 0, p_), qk_steps(0, p_, 1), MM_QK)
            push_unit(("qk", 0, p_), qk_steps(0, KT + p_, 1), MM_QK)
            if p_ <= 4:
                # spread b1's v(j=0) production across b0's windows
                rt0 = (p_ - 1) * 2
                push_unit(("v", 1, 0), v_steps(1, rt0, 0), MM_VP)
                push_unit(("v", 1, 0), v_steps(1, rt0 + 1, 0), MM_VP)
        push_unit(("qk", 1, 0), qk_steps(1, 0, 0), MM_QK)
        push_unit(("qk", 1, 0), qk_steps(1, KT, 0), MM_QK)
        push_unit(("qk", 1, 0), qk_steps(1, 0, 1), MM_QK)
        push_unit(("qk", 1, 0), qk_steps(1, KT, 1), MM_QK)

        for p in range(NP):
            attn_pair(0, p, pre_iter=pre_iter_p0 if p == 0 else None)

        # b1-window fillers: remaining qk, b1's v(j=1), b0's projection
        for p_ in range(1, NP):
            push_unit(("qk", 1, p_), qk_steps(1, p_, 0), MM_QK)
            push_unit(("qk", 1, p_), qk_steps(1, KT + p_, 0), MM_QK)
            push_unit(("qk", 1, p_), qk_steps(1, p_, 1), MM_QK)
            push_unit(("qk", 1, p_), qk_steps(1, KT + p_, 1), MM_QK)
            if p_ <= 4:
                rt0 = (p_ - 1) * 2
                push_unit(("v", 1, 1), v_steps(1, rt0, 1), MM_VP)
                push_unit(("v", 1, 1), v_steps(1, rt0 + 1, 1), MM_VP)
        for rt in range(NKT):
            push_unit(("proj", 0), proj_steps(0, rt), MM_VP)

        def pre_iter_last(hf, kt):
            # b1 projection first half inside the last attention window
            if hf == 1 and 2 <= kt <= 5:
                emit_unit(proj_steps(1, kt - 2))

        for p in range(NP):
            attn_pair(1, p, pre_iter=pre_iter_last if p == NP - 1 else None)
        force_drain()
        pump_pv(force=True)
        for rt in range(4, NKT):
            emit_unit(proj_steps(1, rt))

    nc.compile()
    return nc


_NC_CACHE = None


def _get_nc():
    global _NC_CACHE
    if _NC_CACHE is None:
        _NC_CACHE = build_kernel()
    return _NC_CACHE


def make_in_maps(x, W_qkv, W_proj, b_proj):
    x = np.asarray(x, np.float32)
    wq_full = np.asarray(W_qkv, np.float32)
    # q|k columns -> [p, nt, kt, c] layout, flattened to [128, 9216]
    wqk_r = wq_full[:, :2 * D].reshape(KT, P, 2 * KT, P).transpose(1, 2, 0, 3)
    wqk_host = np.ascontiguousarray(wqk_r.reshape(P, 2 * KT * KT * P)).astype(
        ml_dtypes.bfloat16)
    wv_host = np.ascontiguousarray(wq_full[:, 2 * D:]).astype(ml_dtypes.bfloat16)
    wp = np.asarray(W_proj, np.float32).astype(ml_dtypes.bfloat16)
    bias = np.ascontiguousarray(
        np.broadcast_to(np.asarray(b_proj, np.float32), (P, D))
    )
    in_maps = []
    for c in range(NCORES):
        xc = x[BPC * c:BPC * (c + 1)].reshape(ROWS, D).T
        in_maps.append({
            "xT": np.ascontiguousarray(xc).astype(ml_dtypes.bfloat16),
            "wqk": wqk_host, "wv": wv_host, "wproj": wp, "bias": bias,
        })
    return in_maps


def run(x, W_qkv, W_proj, b_proj, trace=False):
    nc = _get_nc()
    in_maps = make_in_maps(x, W_qkv, W_proj, b_proj)
    res = run_bass_kernel_spmd(nc, in_maps, core_ids=list(range(NCORES)), trace=trace)
    y = np.concatenate(
        [res.results[c]["out"].reshape(BPC, N, D) for c in range(NCORES)], axis=0
    )
    return y.astype(np.float32), res


def kernel(x, W_qkv, W_proj, b_proj):
    y, _ = run(x, W_qkv, W_proj, b_proj, trace=False)
    return y
